# revision 1
# baseline (speedup 1.0000x reference)
"""GAT (3-layer graph attention network) Trainium2 Bass kernel.

Problem: nn_GAT (B=8 graphs, N=1024 nodes, dense adjacency).
Sharding: data-parallel over batch — one graph per NeuronCore, no collectives.

Algorithm notes (per core):
  The GAT attention field alpha[i,j,h] = softmax_j(leakyrelu(s_j + d_i) masked)
  is computed via the identity
      exp(leakyrelu(x)) = max(e^x, e^{0.2 x})
  which factors rank-1:
      e[i,j]/v_i = max(P'_j * Q_i, u_j),  P' = e^s, u = e^{0.2 s}, Q = e^{0.8 d}
  (the per-destination factor v_i = e^{0.2 d_i} cancels in softmax normalization).
  The masked field eg[j,i] = max(P'_j Q_i, u_j) * mask_T[j,i] is built in
  [j(partition), i(free)] layout with ONE fused tensor_scalar (two per-partition
  scalars) + ONE tensor_tensor per tile (bf16, DVE 4x/2x modes), then fed
  straight to the PE as the aggregation matmul's streaming operand.  A
  ones-column appended to the h operand produces the softmax denominator in the
  same matmul.  Layer outputs are produced directly in transposed
  [feature, node] orientation so they feed the next layer's transform (run as
  float32r matmuls — full PE speed at N=512, near-fp32 precision) without
  extra transposes.  Normalization runs in fp32 on GPSIMD.
"""
import numpy as np

B, N, FIN, C, OUT = 8, 1024, 64, 64, 64
P = 128
NT = N // P  # 8 node tiles

_CACHE = {}


def _build(loop=None, debug=False):
    import concourse.bass as bass
    import concourse.mybir as mybir
    import concourse.tile as tile
    from concourse import bacc
    from concourse.masks import make_identity

    fp32 = mybir.dt.float32
    f32r = mybir.dt.float32r
    bf16 = mybir.dt.bfloat16
    i32 = mybir.dt.int32
    OP = mybir.AluOpType
    AT = mybir.ActivationFunctionType

    nc = bacc.Bacc(None, target_bir_lowering=False)

    x0_d = nc.dram_tensor("node_features", [N, FIN], fp32, kind="ExternalInput")
    adj_d = nc.dram_tensor("adj", [N, N], i32, kind="ExternalInput")
    w_d = {}
    for nm, shp in (("w1", [256, 64]), ("as1", [4, 64]), ("ad1", [4, 64]), ("b1", [256]),
                    ("w2", [256, 256]), ("as2", [4, 64]), ("ad2", [4, 64]), ("b2", [256]),
                    ("w3", [64, 256]), ("as3", [1, 64]), ("ad3", [1, 64]), ("b3", [64]),
                    ("wn", [64, 64]), ("bn", [64]), ("wg", [64, 64]), ("bg", [64]),
                    ("wv", [1, 128]), ("bv", [1])):
        w_d[nm] = nc.dram_tensor(nm, shp, fp32, kind="ExternalInput")
    y_d = nc.dram_tensor("out", [1, N], fp32, kind="ExternalOutput")
    dbg_d = {}
    if debug:
        import concourse.mybir as _mb
        for nm, shp, dt in (("dbg_mask", [P, N], bf16), ("dbg_xT0", [64, N], fp32),
                            ("dbg_hT0", [P, N], fp32), ("dbg_srow", [4, N], fp32),
                            ("dbg_qrow", [4, N], bf16), ("dbg_den", [4, N], fp32),
                            ("dbg_x1", [P, N], fp32), ("dbg_eg", [P, N], bf16),
                            ("dbg_ahat", [P, 8], fp32)):
            dbg_d[nm] = nc.dram_tensor(nm, shp, dt, kind="ExternalOutput")

    # DRAM scratch for row-broadcast roundtrips
    qscr = nc.dram_tensor("qscr", [4, N], bf16)
    rscr = nc.dram_tensor("rscr", [4, N], fp32)

    with tile.TileContext(nc) as tc:
        import contextlib
        ctx = contextlib.ExitStack()
        with ctx:
            _pp = ctx.enter_context(tc.tile_pool(name="pp", bufs=1))
            _stg = ctx.enter_context(tc.tile_pool(name="stg", bufs=2))
            _fld = ctx.enter_context(tc.tile_pool(name="fld", bufs=3))
            _ps = ctx.enter_context(tc.tile_pool(name="ps", bufs=2, space="PSUM"))

            class _PoolWrap:
                def __init__(self, p):
                    self.p = p

                def tile(self, shape, dtype, tag, bufs=None):
                    return self.p.tile(shape, dtype, name=tag, tag=tag, bufs=bufs)

            pp, stg, fld, ps = (_PoolWrap(p) for p in (_pp, _stg, _fld, _ps))

            if loop:
                ctx.enter_context(tc.For_i(0, loop, 1))

            def r_(ap):
                return ap.bitcast(f32r)

            def dram_bcast(dram_ap, parts, width):
                return bass.AP(tensor=dram_ap.tensor, offset=dram_ap.offset,
                               ap=[[0, parts], [1, width]])

            # ---------------- identities ----------------
            identf = pp.tile([P, P], fp32, tag="identf")
            make_identity(nc, identf)
            identb = pp.tile([P, P], bf16, tag="identb")
            nc.vector.tensor_copy(out=identb, in_=identf)
            identr = pp.tile([P, P], f32r, tag="identr")
            nc.scalar.copy(out=identr, in_=identf)

            # ---------------- x0 -> xT0 [64, N] f32 ----------------
            xT0 = pp.tile([FIN, N], f32r, tag="xT0")
            xs = stg.tile([P, NT * FIN], fp32, tag="xs")
            nc.sync.dma_start(out=xs.rearrange("p (k f) -> p k f", f=FIN),
                              in_=x0_d.rearrange("(k p) f -> p k f", p=P))
            for k in range(NT):
                pt = ps.tile([FIN, P], fp32, tag="tr")
                nc.tensor.transpose(pt, xs[:, k * FIN:(k + 1) * FIN], identf)
                nc.scalar.copy(out=xT0[:, k * P:(k + 1) * P], in_=pt)

            # ---------------- mask build: maskT[t][j_loc, i] = adj_sl[i, t*128+j_loc]
            maskT = []
            for t in range(NT):
                maskT.append(pp.tile([P, N], bf16, tag=f"maskT{t}"))
            for g in range(4):
                astage = stg.tile([P, 2 * N], i32, tag="adjstg")
                nc.sync.dma_start(
                    out=astage.rearrange("p (k j) -> p k j", j=2 * P),
                    in_=adj_d[:, g * 2 * P:(g + 1) * 2 * P]
                    .rearrange("(k p) j -> p k j", p=P))
                sgn = stg.tile([P, 2 * N], bf16, tag="sgn")
                nc.gpsimd.tensor_copy(out=sgn, in_=astage)
                for tl in range(2):
                    t = 2 * g + tl
                    for kh in range(2):
                        pt = ps.tile([P, 512], bf16, tag="tr")
                        for k4 in range(4):
                            k = kh * 4 + k4
                            nc.tensor.transpose(
                                pt[:, k4 * P:(k4 + 1) * P],
                                sgn[:, k * 2 * P + tl * P:k * 2 * P + (tl + 1) * P],
                                identb)
                        nc.scalar.copy(
                            out=maskT[t][:, kh * 512:(kh + 1) * 512], in_=pt)
                    # self-loops on the diagonal block
                    nc.vector.tensor_tensor(
                        out=maskT[t][:, t * P:(t + 1) * P],
                        in0=maskT[t][:, t * P:(t + 1) * P], in1=identb, op=OP.max)

            if debug:
                nc.sync.dma_start(out=dbg_d["dbg_mask"][:, :], in_=maskT[0])
                nc.sync.dma_start(out=dbg_d["dbg_xT0"][:, :], in_=xT0.bitcast(fp32))
            # ---------------- transposed weights (f32) ----------------
            def build_wT(dram, R, Cdim, tag, dt_=None):
                dt_ = dt_ or f32r
                nk = (Cdim + P - 1) // P
                tiles = []
                for kk in range(nk):
                    kr = min(P, Cdim - kk * P)
                    tiles.append(pp.tile([kr, R], dt_, tag=f"{tag}_{kk}"))
                nm = (R + P - 1) // P
                for mm in range(nm):
                    mr = min(P, R - mm * P)
                    wst = stg.tile([mr, Cdim], fp32, tag="wst")
                    nc.sync.dma_start(out=wst, in_=dram[mm * P:mm * P + mr, :])
                    for kk in range(nk):
                        kr = min(P, Cdim - kk * P)
                        pw = ps.tile([kr, mr], fp32, tag="tr")
                        nc.tensor.transpose(
                            pw, wst[:, kk * P:kk * P + kr], identf[0:mr, 0:mr])
                        nc.scalar.copy(out=tiles[kk][:, mm * P:mm * P + mr], in_=pw)
                return tiles

            W1T = build_wT(w_d["w1"], 256, 64, "w1T")      # [ [64,256] ]
            W2T = build_wT(w_d["w2"], 256, 256, "w2T")     # [ [128,256] x2 ]
            W3T = build_wT(w_d["w3"], 64, 256, "w3T")      # [ [128,64] x2 ]
            WNT = build_wT(w_d["wn"], 64, 64, "wnT", dt_=fp32)[0]    # [64,64]
            WGT = build_wT(w_d["wg"], 64, 64, "wgT", dt_=fp32)[0]    # [64,64]

            # wv columns: wvc[:,0] = wv[0,:64], wvc[:,1] = wv[0,64:]
            wv_sb = stg.tile([1, 128], fp32, tag="wvs")
            nc.sync.dma_start(out=wv_sb, in_=w_d["wv"][:, :])
            wvc = pp.tile([64, 2], fp32, tag="wvc")
            for i in range(2):
                pw = ps.tile([64, 1], fp32, tag="tr")
                nc.tensor.transpose(pw, wv_sb[:, i * 64:(i + 1) * 64], identf[0:1, 0:1])
                nc.scalar.copy(out=wvc[:, i:i + 1], in_=pw)

            # attention vectors -> block-diag Ahat [krows, nk*H] f32
            def build_ahat(dram, H, Fdim, tag):
                nk = (Fdim + P - 1) // P
                krows = min(P, Fdim)
                ahat = pp.tile([krows, nk * H], f32r, tag=tag)
                pah = ps.tile([krows, nk * H], fp32, tag="tr")
                for col in range(nk * H):
                    kk, h = col // H, col % H
                    row = stg.tile([1, krows], fp32, tag="arow")
                    nc.vector.memset(row, 0.0)
                    if (h * 64) // P == kk:
                        po = (h * 64) % P
                        nc.sync.dma_start(out=row[0:1, po:po + 64], in_=dram[h:h + 1, :])
                    nc.tensor.transpose(pah[:, col:col + 1], row, identf[0:1, 0:1])
                nc.scalar.copy(out=ahat, in_=pah)
                return ahat

            def build_CT(ahat, wdram, H, HCdim, Fdim, tag):
                nkc = (HCdim + P - 1) // P
                pc = ps.tile([H, Fdim], fp32, tag="sds", bufs=1)
                for kk in range(nkc):
                    kr = min(P, HCdim - kk * P)
                    wns = stg.tile([kr, Fdim], fp32, tag="wnats")
                    nc.sync.dma_start(out=wns, in_=wdram[kk * P:kk * P + kr, :])
                    wn_ = stg.tile([kr, Fdim], f32r, tag="wnat")
                    nc.scalar.copy(out=wn_, in_=wns)
                    nc.tensor.matmul(pc, lhsT=ahat[0:kr, kk * H:(kk + 1) * H],
                                     rhs=wn_, start=(kk == 0), stop=(kk == nkc - 1))
                crow = stg.tile([H, Fdim], fp32, tag="crow")
                nc.scalar.copy(out=crow, in_=pc)
                nfc = (Fdim + P - 1) // P
                tiles = []
                for ff in range(nfc):
                    fr = min(P, Fdim - ff * P)
                    ct = pp.tile([fr, H], f32r, tag=f"{tag}_{ff}")
                    ptc = ps.tile([fr, H], fp32, tag="tr")
                    nc.tensor.transpose(ptc, crow[:, ff * P:ff * P + fr],
                                        identf[0:H, 0:H])
                    nc.scalar.copy(out=ct, in_=ptc)
                    tiles.append(ct)
                return tiles

            AS1 = build_ahat(w_d["as1"], 4, 256, "as1h")
            if debug:
                nc.sync.dma_start(out=dbg_d["dbg_ahat"][:, :], in_=AS1.bitcast(fp32))
            AD1 = build_ahat(w_d["ad1"], 4, 256, "ad1h")
            AS2 = build_ahat(w_d["as2"], 4, 256, "as2h")
            AD2 = build_ahat(w_d["ad2"], 4, 256, "ad2h")
            AS3 = build_ahat(w_d["as3"], 1, 64, "as3h")
            AD3 = build_ahat(w_d["ad3"], 1, 64, "ad3h")


            # bias columns
            def bias_cols(dram, R, tag):
                cols = []
                for kk in range((R + P - 1) // P):
                    kr = min(P, R - kk * P)
                    t_ = pp.tile([kr, 1], fp32, tag=f"{tag}_{kk}")
                    nc.sync.dma_start(
                        out=t_, in_=dram[kk * P:kk * P + kr].rearrange("(p o) -> p o", o=1))
                    cols.append(t_)
                return cols

            B1 = bias_cols(w_d["b1"], 256, "b1c")
            B2 = bias_cols(w_d["b2"], 256, "b2c")
            B3 = bias_cols(w_d["b3"], 64, "b3c")
            BN = bias_cols(w_d["bn"], 64, "bnc")
            BG = bias_cols(w_d["bg"], 64, "bgc")
            bv_sb = pp.tile([1, 1], fp32, tag="bvc")
            nc.sync.dma_start(out=bv_sb, in_=w_d["bv"].rearrange("(p o) -> p o", o=1))

            # ---------------- GAT layer ----------------
            def gat_layer(li, H, xin, WT, AS, AD, BL, out_dt=None):
                """xin: list of K-chunk tiles [kr, N] f32. Returns list of out chunks."""
                HC = H * 64
                nmch = (HC + P - 1) // P
                nk = len(xin)

                # 1. transform (f32r matmuls): hT[m] [mr, N] f32
                hT = []
                for m in range(nmch):
                    mr = min(P, HC - m * P)
                    hT.append(pp.tile([mr, N], f32r, tag=f"hT{m}"))
                for m in range(nmch):
                    mr = min(P, HC - m * P)
                    for half in range(2):
                        pm = ps.tile([mr, 512], fp32, tag="tfm", bufs=1)
                        for kk in range(nk):
                            nc.tensor.matmul(
                                pm, lhsT=WT[kk][:, m * P:m * P + mr],
                                rhs=xin[kk][:, half * 512:(half + 1) * 512],
                                start=(kk == 0), stop=(kk == nk - 1))
                        nc.scalar.copy(out=hT[m][:, half * 512:(half + 1) * 512], in_=pm)

                if debug and li == 1:
                    nc.sync.dma_start(out=dbg_d["dbg_hT0"][:, :], in_=hT[0].bitcast(fp32))
                # 2. s rows (f32) and Q rows = exp(0.8 d) (bf16)
                srow = pp.tile([H, N], fp32, tag="srow")
                Qrow = pp.tile([H, N], bf16, tag="qrow")
                for (ahat, dst, scl) in ((AD, Qrow, 0.8), (AS, srow, None)):
                    for half in range(2):
                        pv = ps.tile([H, 512], fp32, tag="sds", bufs=1)
                        for kk in range(nmch):
                            kr = hT[kk].shape[0]
                            nc.tensor.matmul(
                                pv, lhsT=ahat[0:kr, kk * H:(kk + 1) * H],
                                rhs=hT[kk][:, half * 512:(half + 1) * 512],
                                start=(kk == 0), stop=(kk == nmch - 1))
                        if scl is None:
                            nc.scalar.copy(out=dst[:, half * 512:(half + 1) * 512], in_=pv)
                        else:
                            nc.scalar.activation(
                                out=dst[:, half * 512:(half + 1) * 512], in_=pv,
                                func=AT.Exp, scale=scl)

                if debug and li == 1:
                    nc.sync.dma_start(out=dbg_d["dbg_srow"][0:H, :], in_=srow)
                    nc.sync.dma_start(out=dbg_d["dbg_qrow"][0:H, :], in_=Qrow)
                # 3. s columns -> P' = e^s, u = e^{0.2 s} per node tile (f32)
                pus = ps.tile([P, NT * H], fp32, tag="sds", bufs=1)
                for jt in range(NT):
                    nc.tensor.transpose(pus[:, jt * H:(jt + 1) * H],
                                        srow[:, jt * P:(jt + 1) * P],
                                        identf[0:H, 0:H])
                puall = pp.tile([P, 2 * NT * H], fp32, tag="puall")
                nc.scalar.activation(out=puall[:, 0:NT * H], in_=pus,
                                     func=AT.Exp, scale=1.0)
                nc.scalar.activation(out=puall[:, NT * H:2 * NT * H], in_=pus,
                                     func=AT.Exp, scale=0.2)

                # 4. Q broadcast tiles via DRAM roundtrip (bf16), one DMA
                nc.sync.dma_start(out=qscr[0:H, :], in_=Qrow)
                qbball = pp.tile([P, H * N], bf16, tag="qbball")
                nc.sync.dma_start(out=qbball,
                                  in_=dram_bcast(qscr[0:1, 0:1], P, H * N))
                Qbb = [qbball[:, h * N:(h + 1) * N] for h in range(H)]

                # 5. h1 tiles: [P, H*65] bf16 = per-head [64 features | 1.0]
                h1 = []
                for jt in range(NT):
                    t_ = pp.tile([P, H * 65], bf16, tag=f"h1_{jt}")
                    nc.gpsimd.memset(t_, 1.0)
                    h1.append(t_)
                for jt in range(NT):
                    for m in range(nmch):
                        mr = hT[m].shape[0]
                        hpc = mr // 64
                        pt = ps.tile([P, mr], f32r, tag="tr")
                        nc.tensor.transpose(pt, hT[m][:, jt * P:(jt + 1) * P],
                                            identr[0:mr, 0:mr])
                        ov = h1[jt].rearrange("p (h c) -> p h c", c=65)
                        nc.scalar.copy(
                            out=ov[:, m * 2:m * 2 + hpc, 0:64],
                            in_=pt.rearrange("p (h c) -> p h c", c=64))

                # 6+7. attention field (bf16 DVE) + aggregation (bf16 PE)
                numall = stg.tile([65, 8 * 512], fp32, tag="numall")
                nums = []
                for h in range(H):
                    pso = [ps.tile([65, 512], fp32, tag=f"agg{half}")
                           for half in range(2)]
                    for jt in range(NT):
                        g = fld.tile([P, N], bf16, tag="g")
                        nc.vector.tensor_scalar(
                            out=g, in0=Qbb[h],
                            scalar1=puall[:, jt * H + h:jt * H + h + 1],
                            scalar2=puall[:, NT * H + jt * H + h:
                                          NT * H + jt * H + h + 1],
                            op0=OP.mult, op1=OP.max)
                        eg = fld.tile([P, N], bf16, tag="eg")
                        tt_eng = nc.gpsimd if (H > 1 and h == 1) else nc.vector
                        tt_eng.tensor_tensor(out=eg, in0=g, in1=maskT[jt], op=OP.mult)
                        if debug and li == 1 and h == 0 and jt == 0:
                            nc.sync.dma_start(out=dbg_d["dbg_eg"][:, :], in_=eg)
                        for half in range(2):
                            nc.tensor.matmul(
                                pso[half], lhsT=h1[jt][:, h * 65:h * 65 + 65],
                                rhs=eg[:, half * 512:(half + 1) * 512],
                                start=(jt == 0), stop=(jt == NT - 1))
                    hnum = []
                    for half in range(2):
                        seg = (h * 2 + half) * 512
                        nc.scalar.copy(out=numall[:, seg:seg + 512], in_=pso[half])
                        hnum.append(numall[:, seg:seg + 512])
                    nums.append(hnum)

                # 8. batched normalize (f32, divide on gpsimd) + bias + relu
                xout = []
                for m in range(nmch):
                    mr = min(P, HC - m * P)
                    xout.append(pp.tile([mr, N], out_dt or f32r, tag=f"xo{li}_{m}"))

                den_pr = [pp.tile([2, N], fp32, tag=f"densb{pr}")
                          for pr in range((H + 1) // 2)]
                rec_pr = [pp.tile([2, N], fp32, tag=f"recsb{pr}")
                          for pr in range((H + 1) // 2)]
                for h in range(H):
                    nc.sync.dma_start(out=den_pr[h // 2][h % 2:h % 2 + 1, :],
                                      in_=numall[64:65, h * 1024:(h + 1) * 1024])
                for pr in range((H + 1) // 2):
                    hs = min(2, H - pr * 2)
                    nc.vector.reciprocal(out=rec_pr[pr][0:hs, :],
                                         in_=den_pr[pr][0:hs, :])
                    rflat = bass.AP(tensor=rscr[0:1, 0:1].tensor,
                                    offset=pr * 2 * 1024,
                                    ap=[[1024, hs], [1, 1024]])
                    nc.sync.dma_start(out=rflat, in_=rec_pr[pr][0:hs, :])
                    dbb = stg.tile([64, 2 * N], fp32, tag="dbb")
                    src_b = bass.AP(tensor=rscr[0:1, 0:1].tensor,
                                    offset=pr * 2 * 1024,
                                    ap=[[0, 64], [1, hs * 1024]])
                    nc.sync.dma_start(out=dbb[:, 0:hs * 1024], in_=src_b)
                    for hh in range(hs):
                        h = pr * 2 + hh
                        m, po = h // 2, (h % 2) * 64
                        for half in range(2):
                            tn = stg.tile([64, 512], fp32, tag="tnorm")
                            nc.gpsimd.tensor_tensor(
                                out=tn, in0=nums[h][half][0:64, :],
                                in1=dbb[:, hh * 1024 + half * 512:
                                        hh * 1024 + (half + 1) * 512],
                                op=OP.mult)
                            nc.scalar.activation(
                                out=xout[m][po:po + 64, half * 512:(half + 1) * 512],
                                in_=tn, func=AT.Relu,
                                bias=BL[m][po:po + 64, 0:1], scale=1.0)
                return xout

            x1 = gat_layer(1, 4, [xT0], W1T, AS1, AD1, B1)
            if debug:
                nc.sync.dma_start(out=dbg_d["dbg_x1"][:, :], in_=x1[0].bitcast(fp32))
            x2 = gat_layer(2, 4, x1, W2T, AS2, AD2, B2)
            x3 = gat_layer(3, 1, x2, W3T, AS3, AD3, B3, out_dt=fp32)
            x3T = x3[0]  # [64, N] f32

            # ---------------- final MLP (f32 / f32r) ----------------
            reluA = pp.tile([64, N], fp32, tag="reluA")
            for half in range(2):
                pA = ps.tile([64, 512], fp32, tag="tfm", bufs=1)
                nc.tensor.matmul(pA, lhsT=WNT,
                                 rhs=x3T[:, half * 512:(half + 1) * 512],
                                 start=True, stop=True)
                nc.scalar.activation(out=reluA[:, half * 512:(half + 1) * 512],
                                     in_=pA, func=AT.Relu, bias=BN[0], scale=1.0)
            gcol = pp.tile([64, 1], fp32, tag="gcol")
            nc.vector.reduce_sum(out=gcol, in_=x3T, axis=mybir.AxisListType.X)
            pg = ps.tile([64, 1], fp32, tag="tr")
            nc.tensor.matmul(pg, lhsT=WGT, rhs=gcol, start=True, stop=True)
            grelu = pp.tile([64, 1], fp32, tag="grelu")
            nc.scalar.activation(out=grelu, in_=pg, func=AT.Relu, bias=BG[0], scale=1.0)
            pk = ps.tile([1, 1], fp32, tag="tr")
            nc.tensor.matmul(pk, lhsT=grelu, rhs=wvc[:, 1:2], start=True, stop=True)
            kap = pp.tile([1, 1], fp32, tag="kap")
            nc.scalar.copy(out=kap, in_=pk)
            ysb = pp.tile([1, N], fp32, tag="ysb")
            for half in range(2):
                py = ps.tile([1, 512], fp32, tag="agg0")
                nc.tensor.matmul(py, lhsT=wvc[:, 0:1],
                                 rhs=reluA[:, half * 512:(half + 1) * 512],
                                 start=True, stop=True)
                nc.vector.tensor_scalar(
                    out=ysb[:, half * 512:(half + 1) * 512], in0=py,
                    scalar1=kap[0:1, 0:1], scalar2=bv_sb[0:1, 0:1],
                    op0=OP.add, op1=OP.add)
            nc.sync.dma_start(out=y_d[:, :], in_=ysb)

    nc.compile()
    return nc


def _get_prog():
    if "nc" not in _CACHE:
        _CACHE["nc"] = _build()
    return _CACHE["nc"]


def kernel(**inputs):
    from concourse.bass_utils import run_bass_kernel_spmd

    nc = _get_prog()
    names = ["w1", "as1", "ad1", "b1", "w2", "as2", "ad2", "b2",
             "w3", "as3", "ad3", "b3", "wn", "bn", "wg", "bg", "wv", "bv"]
    in_maps = []
    for b in range(B):
        m = {"node_features": np.ascontiguousarray(inputs["node_features"][b]),
             "adj": np.ascontiguousarray(inputs["adj"][b])}
        for nm in names:
            m[nm] = np.ascontiguousarray(inputs[nm], dtype=np.float32)
        in_maps.append(m)
    res = run_bass_kernel_spmd(nc, in_maps, list(range(B)))
    out = np.stack([res.results[b]["out"][0] for b in range(B)], axis=0)
    return out.astype(np.float32)


if __name__ == "__main__":
    nc = _get_prog()
    print("build ok")

